# revision 1
# baseline (speedup 1.0000x reference)
"""GNN message-passing layer (LplsNorm + residual conv) on 8 Trainium2 cores.

Computation (reference, all f32):
    degree = A.sum(-1); ds = degree**-0.5
    mf  = f + ds[:,None] * (A @ (ds[:,None] * f))      # a_norm = ds A ds
    out = relu(mf @ W + b)

Distribution: A row-sharded over 8 cores ([1024, 8192] each), feature
replicated.

Per-core schedule (v3):
  - Single streaming pass over the A shard. Per [128, 2048] chunk: ScalarE
    accumulates row sums (degree, f32-exact), GpSimd casts the chunk to
    bf16, TensorE transposes the 16 [128,128] bf16 tiles (cheap LDW), DVE
    copies them out of PSUM. 7/16ths stay resident in SBUF; the rest spill
    to a bf16 DRAM scratch. Keeps the PE warm and off the f32 weight-load
    path.
  - Tiny AllGather shares per-core degree; ds = 1/sqrt(degree) via DVE
    reciprocal + ACT sqrt.
  - X' = ds * f cast to bf16, produced in 1 MiB batches.
  - Main matmul runs kc-outer over groups of 4 m-tiles (4 PSUM
    accumulators), so the PE saturates as soon as the first X' chunks
    appear instead of being paced by X' production.
  - Epilogue per m-tile: mf = Y * ds_own + f_res (fused DVE op), mf @ W in
    f32r (full-rate PE), bias via a K=1 matmul with a ones row, ACT relu.
"""

import numpy as np

import concourse.bass as bass
import concourse.mybir as mybir
import concourse.tile as tile
from concourse import bacc
from concourse import bass_utils
from concourse.masks import make_identity

N = 8192
D = 512
NCORES = 8
P = 128
R = N // NCORES          # rows per core: 1024
MT = R // P              # m-tiles per core: 8
KC = N // P              # k-chunks: 64
ACH = 2048               # A stream chunk width (f32 -> 1 MiB per DMA)
NACH = N // ACH          # stream chunks per row-block: 4
GPC = ACH // (4 * P)     # transpose groups (of 4 tiles) per stream chunk: 4
NG = KC // 4             # k-groups total: 16
NG_RES = 7               # k-groups resident in SBUF (kc 0..27)
MTG = 4                  # m-tiles per matmul group (PSUM accumulators)

F32 = mybir.dt.float32
F32R = mybir.dt.float32r
BF16 = mybir.dt.bfloat16

_NC_CACHE = {}


def _build():
    nc = bacc.Bacc("TRN2", target_bir_lowering=False, debug=False, num_devices=NCORES)

    a_d = nc.dram_tensor("a", [R, N], F32, kind="ExternalInput")
    f_d = nc.dram_tensor("f", [N, D], F32, kind="ExternalInput")
    fres_d = nc.dram_tensor("fres", [R, D], F32, kind="ExternalInput")
    w_d = nc.dram_tensor("w", [D, D], F32R, kind="ExternalInput")
    b_d = nc.dram_tensor("bias", [1, D], F32, kind="ExternalInput")
    out_d = nc.dram_tensor("out", [R, D], F32, kind="ExternalOutput")

    AX = mybir.AxisListType.X
    ALU = mybir.AluOpType
    ACT = mybir.ActivationFunctionType

    with tile.TileContext(nc) as tc:
        with (
            tc.tile_pool(name="const", bufs=1) as constp,
            tc.tile_pool(name="deg", bufs=1) as degp,
            tc.tile_pool(name="astream", bufs=3) as astreamp,
            tc.tile_pool(name="small", bufs=2) as smallp,
            tc.tile_pool(name="atres", bufs=1) as atresp,
            tc.tile_pool(name="atw", bufs=2) as atwp,
            tc.tile_pool(name="xp", bufs=1) as xpp,
            tc.tile_pool(name="fstream", bufs=2) as fstreamp,
            tc.tile_pool(name="epi", bufs=2) as epip,
            tc.tile_pool(name="mft", bufs=2) as mftp,
            tc.tile_pool(name="psA", bufs=2, space="PSUM") as psA,      # transpose groups
            tc.tile_pool(name="psY", bufs=MTG, space="PSUM") as psY,    # Y accumulators
            tc.tile_pool(name="psaux", bufs=1, space="PSUM") as psaux,  # small transposes
            tc.tile_pool(name="psO", bufs=1, space="PSUM") as psO,      # second matmul out
            tc.tile_pool(name="dram", bufs=1, space="DRAM") as dramp,
        ):
            # ---- constants ----
            identity = constp.tile([P, P], F32)
            make_identity(nc, identity[:])
            identity_bf = constp.tile([P, P], BF16)
            make_identity(nc, identity_bf[:])
            ones_row = constp.tile([1, P], F32)
            nc.gpsimd.memset(ones_row[:], 1.0)
            b_sb = constp.tile([1, D], F32)
            nc.sync.dma_start(b_sb[:], b_d.ap())
            w_sb = constp.tile([P, 4 * D], F32R)  # w chunk wc at [:, wc*D:(wc+1)*D]
            for wc in range(4):
                nc.sync.dma_start(
                    w_sb[:, wc * D : (wc + 1) * D], w_d.ap()[wc * P : (wc + 1) * P, :]
                )

            # resident transposed-A store: (group g, mt) block at col (g*MT+mt)*4P
            at_res = atresp.tile([P, NG_RES * MT * 4 * P], BF16)
            # DRAM scratch for the non-resident groups
            scratch = dramp.tile([(NG - NG_RES) * MT, P, 4 * P], BF16)
            cin = dramp.tile([MT, P], F32)
            cout = dramp.tile([KC, P], F32)

            # ---- merged pass: degree + transpose-all ----
            degree_sb = degp.tile([P, MT], F32)  # col mt = degree of rows mt*128..
            for mt in range(MT):
                dcols = smallp.tile([P, NACH], F32, tag="dcols")
                for c in range(NACH):
                    ach = astreamp.tile([P, ACH], F32, tag="ach")
                    nc.sync.dma_start(
                        ach[:], a_d.ap()[mt * P : (mt + 1) * P, c * ACH : (c + 1) * ACH]
                    )
                    achb = astreamp.tile([P, ACH], BF16, tag="achb", bufs=2)
                    nc.scalar.activation(
                        achb[:], ach[:], ACT.Copy, accum_out=dcols[:, c : c + 1]
                    )
                    for g in range(GPC):
                        gk = c * GPC + g  # k-group index 0..15
                        trp = psA.tile([P, 4 * P], F32, tag="trp")
                        for q in range(4):
                            nc.tensor.matmul(
                                trp[:, q * P : (q + 1) * P],
                                achb[:, (g * 4 + q) * P : (g * 4 + q + 1) * P],
                                identity_bf[:],
                            )
                        if gk < NG_RES:
                            dst = at_res[
                                :, (gk * MT + mt) * 4 * P : (gk * MT + mt + 1) * 4 * P
                            ]
                        else:
                            dst = atwp.tile([P, 4 * P], BF16, tag="atw")
                        nc.vector.tensor_copy(dst[:], trp[:])
                        if gk >= NG_RES:
                            nc.sync.dma_start(
                                scratch[(gk - NG_RES) * MT + mt], dst[:]
                            )
                nc.vector.reduce_sum(degree_sb[:, mt : mt + 1], dcols[:], axis=AX)

            # ---- AllGather degree ----
            degT_ps = psaux.tile([MT, P], F32, tag="aux")
            nc.tensor.transpose(degT_ps[:], degree_sb[:], identity[:])
            degT_sb = smallp.tile([MT, P], F32, tag="degT")
            nc.vector.tensor_copy(degT_sb[:], degT_ps[:])
            nc.sync.dma_start(cin[:], degT_sb[:])
            nc.gpsimd.collective_compute(
                "AllGather",
                ALU.bypass,
                ins=[cin.opt()],
                outs=[cout.opt()],
                replica_groups=[list(range(NCORES))],
            )
            # cout row g = global degree of rows [g*128, (g+1)*128)
            degall_sb = smallp.tile([KC, P], F32, tag="degall")
            nc.sync.dma_start(degall_sb[:], cout[:])
            degallT_ps = psaux.tile([P, KC], F32, tag="aux")
            nc.tensor.transpose(degallT_ps[:], degall_sb[:], identity[:KC, :KC])
            recip = degp.tile([P, KC], F32)
            nc.vector.reciprocal(recip[:], degallT_ps[:])
            ds_sb = degp.tile([P, KC], F32)  # ds_sb[p, g] = ds[g*128 + p]
            nc.scalar.activation(ds_sb[:], recip[:], ACT.Sqrt)
            recip8 = degp.tile([P, MT], F32)
            nc.vector.reciprocal(recip8[:], degree_sb[:])
            dsown = degp.tile([P, MT], F32)
            nc.scalar.activation(dsown[:], recip8[:], ACT.Sqrt)

            # ---- X' = ds * f, cast to bf16 (1 MiB load batches) ----
            xp_sb = xpp.tile([P, KC * D], BF16)  # chunk kc at [:, kc*D:(kc+1)*D]
            f_blk = f_d.ap().rearrange("(b c p) d -> b p c d", c=4, p=P)
            for fb in range(KC // 4):
                fch = fstreamp.tile([P, 4 * D], F32, tag="fch")
                nc.sync.dma_start(
                    fch[:].rearrange("p (c d) -> p c d", c=4), f_blk[fb]
                )
                for j in range(4):
                    kc = 4 * fb + j
                    nc.vector.tensor_scalar_mul(
                        xp_sb[:, kc * D : (kc + 1) * D],
                        fch[:, j * D : (j + 1) * D],
                        ds_sb[:, kc : kc + 1],
                    )

            # ---- main matmul: kc-outer over groups of MTG m-tiles ----
            for mtg in range(MT // MTG):
                ys = [psY.tile([P, D], F32, tag="y", name=f"y{mtg}_{i}") for i in range(MTG)]
                for gk in range(NG):
                    at4s = []
                    for mi in range(MTG):
                        mt = mtg * MTG + mi
                        if gk < NG_RES:
                            at4 = at_res[
                                :, (gk * MT + mt) * 4 * P : (gk * MT + mt + 1) * 4 * P
                            ]
                        else:
                            at4t = atwp.tile([P, 4 * P], BF16, tag="atr", bufs=6)
                            nc.sync.dma_start(
                                at4t[:], scratch[(gk - NG_RES) * MT + mt]
                            )
                            at4 = at4t[:]
                        at4s.append(at4)
                    for q in range(4):
                        kc = gk * 4 + q
                        for mi in range(MTG):
                            nc.tensor.matmul(
                                ys[mi][:],
                                at4s[mi][:, q * P : (q + 1) * P],
                                xp_sb[:, kc * D : (kc + 1) * D],
                                start=(kc == 0),
                                stop=(kc == KC - 1),
                            )
                # epilogue per m-tile in the group
                for mi in range(MTG):
                    mt = mtg * MTG + mi
                    res = epip.tile([P, D], F32, tag="res")
                    nc.sync.dma_start(res[:], fres_d.ap()[mt * P : (mt + 1) * P, :])
                    mf = epip.tile([P, D], F32, tag="mf")
                    nc.vector.scalar_tensor_tensor(
                        mf[:],
                        ys[mi][:],
                        dsown[:, mt : mt + 1],
                        res[:],
                        op0=ALU.mult,
                        op1=ALU.add,
                    )
                    o_ps = psO.tile([P, D], F32, tag="o")
                    for wc in range(4):
                        mfT_ps = psaux.tile([P, P], F32, tag="aux")
                        nc.tensor.transpose(
                            mfT_ps[:], mf[:, wc * P : (wc + 1) * P], identity[:]
                        )
                        mfT_sb = mftp.tile([P, P], F32R, tag="mfT")
                        nc.vector.tensor_copy(mfT_sb[:], mfT_ps[:])
                        nc.tensor.matmul(
                            o_ps[:],
                            mfT_sb[:],
                            w_sb[:, wc * D : (wc + 1) * D],
                            start=(wc == 0),
                            stop=False,
                        )
                    nc.tensor.matmul(
                        o_ps[:], ones_row[:], b_sb[:], start=False, stop=True
                    )
                    osb = epip.tile([P, D], F32, tag="osb")
                    nc.scalar.activation(osb[:], o_ps[:], ACT.Relu)
                    nc.sync.dma_start(out_d.ap()[mt * P : (mt + 1) * P, :], osb[:])

    nc.compile()
    return nc


def _get_nc():
    if "nc" not in _NC_CACHE:
        _NC_CACHE["nc"] = _build()
    return _NC_CACHE["nc"]


def run(inputs, trace=False, trace_kwargs=None):
    """Run the SPMD kernel; returns (full_output, BassKernelResults)."""
    a = np.ascontiguousarray(np.asarray(inputs["adjacency_matrix"], dtype=np.float32))
    f = np.ascontiguousarray(np.asarray(inputs["feature"], dtype=np.float32))
    w = np.ascontiguousarray(np.asarray(inputs["W"], dtype=np.float32))
    b = np.ascontiguousarray(np.asarray(inputs["b"], dtype=np.float32)).reshape(1, D)

    nc = _get_nc()
    in_maps = []
    for d in range(NCORES):
        rows = slice(d * R, (d + 1) * R)
        in_maps.append({"a": a[rows], "f": f, "fres": f[rows], "w": w, "bias": b})
    res = bass_utils.run_bass_kernel_spmd(
        nc,
        in_maps,
        core_ids=list(range(NCORES)),
        trace=trace,
        **(trace_kwargs or {}),
    )
    out = np.concatenate([r["out"] for r in res.results], axis=0)
    return out, res


def kernel(**inputs):
    out, _ = run(inputs, trace=False)
    return out



# revision 5
# speedup vs baseline: 1.6873x; 1.6873x over previous
"""GNN message-passing layer (LplsNorm + residual conv) on 8 Trainium2 cores.

Computation (reference, all f32):
    degree = A.sum(-1); ds = degree**-0.5
    mf  = f + ds[:,None] * (A @ (ds[:,None] * f))      # a_norm = ds A ds
    out = relu(mf @ W + b)

Distribution: A row-sharded over 8 cores ([1024, 8192] each), feature
replicated.

Per-core schedule (v4):
  - Single streaming pass over the A shard. Per [128, 2048] f32 chunk:
    ScalarE casts to bf16 + accumulates exact f32 row sums (degree),
    TensorE transposes the 16 [128,128] bf16 tiles via identity matmuls,
    DVE/GpSimd copy them out of PSUM as fp8(e4m3). The ENTIRE transposed
    shard stays SBUF-resident (8 MiB fp8 = 64 KiB/partition) - no DRAM
    scratch at all.
  - Tiny AllGather shares per-core degree; dsq = 64/sqrt(degree) via DVE
    reciprocal + ACT sqrt(x*4096). The x64 exponent boost keeps fp8 X'
    in e4m3's happy range; it is compensated in the epilogue row scale
    (dsown = sqrt(1/deg)/64).
  - X' = dsq * f cast to fp8, produced just-in-time from streamed f32 f
    chunks during the first matmul group (f traffic moves off the
    phase-A critical path).
  - Main matmul in fp8 DoubleRow mode: each instruction contracts K=256
    (two adjacent k-tiles of A^T against two adjacent X' chunks) at 2x
    bf16 rate. kc-outer over 2 groups of 4 m-tiles (4 PSUM accumulators).
  - Epilogue per m-tile: mf = Y * dsown + f_res (fused DVE op), mf @ W in
    f32r (full-rate PE), bias via a K=1 f32r matmul with a ones row,
    ACT relu.
"""

import numpy as np

import concourse.bass as bass
import concourse.mybir as mybir
import concourse.tile as tile
from concourse import bacc
from concourse import bass_utils
from concourse.masks import make_identity

N = 8192
D = 512
NCORES = 8
P = 128
R = N // NCORES          # rows per core: 1024
MT = R // P              # m-tiles per core: 8
KC = N // P              # k-chunks: 64
PAIRS = KC // 2          # DoubleRow k-pairs: 32
ACH = 2048               # A stream chunk width (f32 -> 1 MiB per DMA)
NACH = N // ACH          # stream chunks per row-block: 4
GPC = ACH // (4 * P)     # transpose groups (of 4 tiles) per stream chunk: 4
MTG = 4                  # m-tiles per matmul group (PSUM accumulators)

F32 = mybir.dt.float32
F32R = mybir.dt.float32r
BF16 = mybir.dt.bfloat16
FP8 = mybir.dt.float8e4

_NC_CACHE = {}


def _build():
    nc = bacc.Bacc("TRN2", target_bir_lowering=False, debug=False, num_devices=NCORES)

    a_d = nc.dram_tensor("a", [R, N], F32, kind="ExternalInput")
    f_d = nc.dram_tensor("f", [N, D], F32, kind="ExternalInput")
    fres_d = nc.dram_tensor("fres", [R, D], F32, kind="ExternalInput")
    w_d = nc.dram_tensor("w", [D, D], F32R, kind="ExternalInput")
    b_d = nc.dram_tensor("bias", [1, D], F32R, kind="ExternalInput")
    out_d = nc.dram_tensor("out", [R, D], F32, kind="ExternalOutput")

    AX = mybir.AxisListType.X
    ALU = mybir.AluOpType
    ACT = mybir.ActivationFunctionType
    DR = mybir.MatmulPerfMode.DoubleRow

    with tile.TileContext(nc) as tc:
        with (
            tc.tile_pool(name="const", bufs=1) as constp,
            tc.tile_pool(name="deg", bufs=1) as degp,
            tc.tile_pool(name="astream", bufs=3) as astreamp,
            tc.tile_pool(name="small", bufs=2) as smallp,
            tc.tile_pool(name="atres", bufs=1) as atresp,
            tc.tile_pool(name="xp", bufs=1) as xpp,
            tc.tile_pool(name="fstream", bufs=4) as fstreamp,
            tc.tile_pool(name="epi", bufs=2) as epip,
            tc.tile_pool(name="mft", bufs=2) as mftp,
            tc.tile_pool(name="psA", bufs=2, space="PSUM") as psA,      # transpose groups
            tc.tile_pool(name="psY", bufs=MTG, space="PSUM") as psY,    # Y accumulators
            tc.tile_pool(name="psaux", bufs=1, space="PSUM") as psaux,  # small transposes
            tc.tile_pool(name="psO", bufs=1, space="PSUM") as psO,      # second matmul out
            tc.tile_pool(name="dram", bufs=1, space="DRAM") as dramp,
        ):
            # ---- constants ----
            identity = constp.tile([P, P], F32)
            make_identity(nc, identity[:])
            identity_bf = constp.tile([P, P], BF16)
            make_identity(nc, identity_bf[:])
            ones_row = constp.tile([1, P], F32)
            nc.gpsimd.memset(ones_row[:], 1.0)
            b_sb = constp.tile([1, D], F32R)
            nc.sync.dma_start(b_sb[:], b_d.ap())
            w_sb = constp.tile([P, 4 * D], F32R)  # w chunk wc at [:, wc*D:(wc+1)*D]
            for wc in range(4):
                nc.sync.dma_start(
                    w_sb[:, wc * D : (wc + 1) * D], w_d.ap()[wc * P : (wc + 1) * P, :]
                )

            # fully resident transposed-A store, fp8:
            # m-tile mt's block at [:, mt*KC*P : (mt+1)*KC*P], k-chunk kc at
            # [:, (mt*KC + kc)*P : (mt*KC + kc + 1)*P]
            at_res = atresp.tile([P, MT * KC * P], FP8)
            cin = dramp.tile([MT, P], F32)
            cout = dramp.tile([KC, P], F32)

            # ---- merged pass: degree + transpose-all ----
            degree_sb = degp.tile([P, MT], F32)  # col mt = degree of rows mt*128..
            for mt in range(MT):
                dcols = smallp.tile([P, NACH], F32, tag="dcols")
                for c in range(NACH):
                    ach = astreamp.tile([P, ACH], F32, tag="ach")
                    nc.sync.dma_start(
                        ach[:], a_d.ap()[mt * P : (mt + 1) * P, c * ACH : (c + 1) * ACH]
                    )
                    achb = astreamp.tile([P, ACH], BF16, tag="achb", bufs=2)
                    nc.scalar.activation(
                        achb[:], ach[:], ACT.Copy, accum_out=dcols[:, c : c + 1]
                    )
                    for g in range(GPC):
                        kc0 = c * GPC * 4 + g * 4  # first k-chunk of this group
                        trp = psA.tile([P, 4 * P], F32, tag="trp")
                        for q in range(4):
                            nc.tensor.matmul(
                                trp[:, q * P : (q + 1) * P],
                                achb[:, (g * 4 + q) * P : (g * 4 + q + 1) * P],
                                identity_bf[:],
                            )
                        dst = at_res[:, (mt * KC + kc0) * P : (mt * KC + kc0 + 4) * P]
                        nc.vector.tensor_copy(dst, trp[:])
                nc.vector.reduce_sum(degree_sb[:, mt : mt + 1], dcols[:], axis=AX)

            # ---- AllGather degree ----
            degT_ps = psaux.tile([MT, P], F32, tag="aux")
            nc.tensor.transpose(degT_ps[:], degree_sb[:], identity[:])
            degT_sb = smallp.tile([MT, P], F32, tag="degT")
            nc.vector.tensor_copy(degT_sb[:], degT_ps[:])
            nc.sync.dma_start(cin[:], degT_sb[:])
            nc.gpsimd.collective_compute(
                "AllGather",
                ALU.bypass,
                ins=[cin.opt()],
                outs=[cout.opt()],
                replica_groups=[list(range(NCORES))],
            )
            # cout row g = global degree of rows [g*128, (g+1)*128)
            degall_sb = smallp.tile([KC, P], F32, tag="degall")
            nc.sync.dma_start(degall_sb[:], cout[:])
            degallT_ps = psaux.tile([P, KC], F32, tag="aux")
            nc.tensor.transpose(degallT_ps[:], degall_sb[:], identity[:KC, :KC])
            recip = degp.tile([P, KC], F32)
            nc.vector.reciprocal(recip[:], degallT_ps[:])
            # dsq[p, g] = 64 * ds[g*128 + p]  (x64 fp8 exponent boost)
            dsq = degp.tile([P, KC], F32)
            nc.scalar.activation(dsq[:], recip[:], ACT.Sqrt, scale=4096.0)
            recip8 = degp.tile([P, MT], F32)
            nc.vector.reciprocal(recip8[:], degree_sb[:])
            # dsown[p, mt] = ds[own rows] / 64  (undo the boost)
            dsown = degp.tile([P, MT], F32)
            nc.scalar.activation(dsown[:], recip8[:], ACT.Sqrt, scale=1.0 / 4096.0)

            # X' = dsq * f in fp8; produced during mtg 0 below
            xp_sb = xpp.tile([P, KC * D], FP8)  # chunk kc at [:, kc*D:(kc+1)*D]
            f_blk = f_d.ap().rearrange("(b c p) d -> b p c d", c=4, p=P)

            # ---- main matmul: fp8 DoubleRow, kc-pair-outer, 2 groups of 4 m-tiles ----
            for mtg in range(MT // MTG):
                # prefetch residual rows for this group's epilogue
                ress = []
                for mi in range(MTG):
                    mt = mtg * MTG + mi
                    res = epip.tile([P, D], F32, tag="res", bufs=MTG)
                    nc.sync.dma_start(res[:], fres_d.ap()[mt * P : (mt + 1) * P, :])
                    ress.append(res)
                ys = [
                    psY.tile([P, D], F32, tag="y", name=f"y{mtg}_{i}")
                    for i in range(MTG)
                ]
                for j in range(PAIRS):
                    if mtg == 0 and j % 2 == 0:
                        fb = j // 2  # f block of 4 k-chunks
                        fch = fstreamp.tile([P, 4 * D], F32, tag="fch")
                        nc.sync.dma_start(
                            fch[:].rearrange("p (c d) -> p c d", c=4), f_blk[fb]
                        )
                        for t in range(4):
                            kc = 4 * fb + t
                            nc.vector.tensor_scalar_mul(
                                xp_sb[:, kc * D : (kc + 1) * D],
                                fch[:, t * D : (t + 1) * D],
                                dsq[:, kc : kc + 1],
                            )
                    rhs = xp_sb[:, (2 * j) * D : (2 * j + 2) * D].rearrange(
                        "p (two n) -> p two n", two=2
                    )
                    for mi in range(MTG):
                        mt = mtg * MTG + mi
                        lhsT = at_res[
                            :, (mt * KC + 2 * j) * P : (mt * KC + 2 * j + 2) * P
                        ].rearrange("p (two m) -> p two m", two=2)
                        nc.tensor.matmul(
                            ys[mi][:],
                            lhsT,
                            rhs,
                            start=(j == 0),
                            stop=(j == PAIRS - 1),
                            perf_mode=DR,
                        )
                # epilogue per m-tile in the group
                for mi in range(MTG):
                    mt = mtg * MTG + mi
                    mf = epip.tile([P, D], F32, tag="mf")
                    nc.vector.scalar_tensor_tensor(
                        mf[:],
                        ys[mi][:],
                        dsown[:, mt : mt + 1],
                        ress[mi][:],
                        op0=ALU.mult,
                        op1=ALU.add,
                    )
                    o_ps = psO.tile([P, D], F32, tag="o")
                    for wc in range(4):
                        mfT_ps = psaux.tile([P, P], F32, tag="aux")
                        nc.tensor.transpose(
                            mfT_ps[:], mf[:, wc * P : (wc + 1) * P], identity[:]
                        )
                        mfT_sb = mftp.tile([P, P], F32R, tag="mfT")
                        nc.vector.tensor_copy(mfT_sb[:], mfT_ps[:])
                        nc.tensor.matmul(
                            o_ps[:],
                            mfT_sb[:],
                            w_sb[:, wc * D : (wc + 1) * D],
                            start=(wc == 0),
                            stop=False,
                        )
                    nc.tensor.matmul(
                        o_ps[:], ones_row[:].bitcast(F32R), b_sb[:], start=False, stop=True
                    )
                    osb = epip.tile([P, D], F32, tag="osb")
                    nc.scalar.activation(osb[:], o_ps[:], ACT.Relu)
                    nc.sync.dma_start(out_d.ap()[mt * P : (mt + 1) * P, :], osb[:])

    nc.compile()
    return nc


def _get_nc():
    if "nc" not in _NC_CACHE:
        _NC_CACHE["nc"] = _build()
    return _NC_CACHE["nc"]


def run(inputs, trace=False, trace_kwargs=None):
    """Run the SPMD kernel; returns (full_output, BassKernelResults)."""
    a = np.ascontiguousarray(np.asarray(inputs["adjacency_matrix"], dtype=np.float32))
    f = np.ascontiguousarray(np.asarray(inputs["feature"], dtype=np.float32))
    w = np.ascontiguousarray(np.asarray(inputs["W"], dtype=np.float32))
    b = np.ascontiguousarray(np.asarray(inputs["b"], dtype=np.float32)).reshape(1, D)

    nc = _get_nc()
    in_maps = []
    for d in range(NCORES):
        rows = slice(d * R, (d + 1) * R)
        in_maps.append({"a": a[rows], "f": f, "fres": f[rows], "w": w, "bias": b})
    res = bass_utils.run_bass_kernel_spmd(
        nc,
        in_maps,
        core_ids=list(range(NCORES)),
        trace=trace,
        **(trace_kwargs or {}),
    )
    out = np.concatenate([r["out"] for r in res.results], axis=0)
    return out, res


def kernel(**inputs):
    out, _ = run(inputs, trace=False)
    return out
